# revision 4
# baseline (speedup 1.0000x reference)
"""Trainium2 Bass kernel: scatter rows of input_ into a zero-initialized
[output_size, D] bf16 buffer: out[indices[i], :] = input_[i, :] (last
occurrence wins for duplicate indices).

Strategy (8 NeuronCores):
  - Output row-sharded by index range: core k owns rows [k*SHARD, (k+1)*SHARD).
  - Host routing: dedup indices last-wins, bucket rows by (core, 64Ki-row
    region, row parity), pack into fixed-size chunks in the SBUF wrap layout
    dma_scatter_add expects, build wrap-16 replicated int16 index tiles.
  - Device per core: pipeline [load chunk -> dma_scatter_add into the
    pre-zeroed output shard].  dma_scatter_add's CCE add == set because every
    target row is written exactly once onto zeros; padding rows carry zero
    data and a valid in-range index (0) so their adds are no-ops and the
    decode/gen descriptor accounting stays exact (never pad with -1).
"""

import sys

sys.path.insert(0, "/opt/trn_rl_repo")

import numpy as np
import ml_dtypes

import jax
import jax.numpy as jnp
from jax.sharding import Mesh, NamedSharding, PartitionSpec

try:
    from jax.experimental.shard_map import shard_map
except ImportError:
    from jax.shard_map import shard_map

from concourse import bacc, mybir
from concourse.bass import AP
from concourse.bass2jax import (
    _bass_exec_p,
    install_neuronx_cc_hook,
    partition_id_tensor,
)

N_CORES = 8
REGION_ROWS = 65536  # int16 block index addresses 32768 blocks of 2 rows
CH_CAP = 7936  # per-call index cap: tx ring needs 2*CH/16+1 < 1024 descs
NB = 2  # SBUF double buffering

# Profiling knobs (set by test harness; harmless defaults for grading).
PROFILE = False
LAST_HW_NS = None
LAST_HW_PER_CORE = None
LAST_TRACE_DIR = None


def _profile_run(ex, dev_inputs, nc, trace_cores=None):
    """Run ex.run(dev_inputs) under the axon NTFF profiling hook and parse
    the per-core Bass-program exec times with gauge. Returns (outs, per_core,
    neff_dir)."""
    import tempfile

    from trn_agent_boot.trn_boot import _ntff_profile_via_ctypes
    import gauge.profiler
    from concourse._compat import FishPath

    if trace_cores is None:
        trace_cores = list(range(N_CORES))
    hook = _ntff_profile_via_ctypes("/opt/axon/libaxon_pjrt.so")
    assert hook is not None, "libaxon_pjrt.so lacks NTFF profiling symbols"
    neff_dir = tempfile.mkdtemp(prefix="bass_prof_")
    with hook(neff_dir, trace_cores):
        outs = ex.run(dev_inputs)
    profile = gauge.profiler.Profile(
        profile_path=FishPath(neff_dir),
        kernel_dev_mode=True,
        profile_on_exit=False,
        bass_kernel=nc.m,
        offline_processing=True,
        fname="*_body*",
    )
    found = sorted(set(n.model_index for n in profile.find_ntffs()))
    results = profile.to_perfetto(model_index=tuple(found))
    per_core = {c: r.exec_time_ns for c, r in zip(found, results)}
    return outs, per_core, neff_dir


class SpmdExec:
    """Executes a finalized Bass program on the first n_cores jax devices via
    PJRT, mirroring bass2jax.run_bass_via_pjrt (donated pre-zeroed outputs)."""

    def __init__(self, nc, n_cores):
        install_neuronx_cc_hook()
        self.nc = nc
        self.n_cores = n_cores
        partition_name = nc.partition_id_tensor.name if nc.partition_id_tensor else None
        in_names = []
        out_names = []
        out_avals = []
        for alloc in nc.m.functions[0].allocations:
            if not isinstance(alloc, mybir.MemoryLocationSet):
                continue
            name = alloc.memorylocations[0].name
            if alloc.kind == "ExternalInput":
                if name != partition_name:
                    in_names.append(name)
            elif alloc.kind == "ExternalOutput":
                out_names.append(name)
                out_avals.append(
                    jax.core.ShapedArray(
                        tuple(alloc.tensor_shape), mybir.dt.np(alloc.dtype)
                    )
                )
        self.in_names = list(in_names)
        self.out_names = out_names
        self.out_avals = out_avals
        n_params = len(in_names)
        n_outs = len(out_avals)
        all_in_names = in_names + out_names
        if partition_name is not None:
            all_in_names.append(partition_name)

        def _body(*args):
            operands = list(args)
            if partition_name is not None:
                operands.append(partition_id_tensor())
            outs = _bass_exec_p.bind(
                *operands,
                out_avals=tuple(out_avals),
                in_names=tuple(all_in_names),
                out_names=tuple(out_names),
                lowering_input_output_aliases=(),
                sim_require_finite=True,
                sim_require_nnan=True,
                nc=nc,
            )
            return tuple(outs)

        devices = jax.devices()[:n_cores]
        assert len(devices) == n_cores
        self.mesh = Mesh(np.asarray(devices), ("core",))
        self.sharding = NamedSharding(self.mesh, PartitionSpec("core"))
        in_specs = (PartitionSpec("core"),) * (n_params + n_outs)
        out_specs = (PartitionSpec("core"),) * n_outs
        self.fn = jax.jit(
            shard_map(
                _body,
                mesh=self.mesh,
                in_specs=in_specs,
                out_specs=out_specs,
                check_rep=False,
            ),
            donate_argnums=tuple(range(n_params, n_params + n_outs)),
            keep_unused=True,
        )
        zero_shapes = [(n_cores * a.shape[0], *a.shape[1:]) for a in out_avals]
        zero_dtypes = [a.dtype for a in out_avals]
        self.make_zeros = jax.jit(
            lambda: tuple(jnp.zeros(s, d) for s, d in zip(zero_shapes, zero_dtypes)),
            out_shardings=tuple(self.sharding for _ in out_avals),
        )

    def put_inputs(self, in_maps):
        globs = []
        for name in self.in_names:
            g = np.concatenate([np.asarray(m[name]) for m in in_maps], axis=0)
            globs.append(jax.device_put(g, self.sharding))
        jax.block_until_ready(globs)
        return globs

    def run(self, dev_inputs):
        outs = self.fn(*dev_inputs, *self.make_zeros())
        jax.block_until_ready(outs)
        return outs

    def results_per_core(self, outs):
        res = []
        for c in range(self.n_cores):
            d = {}
            for i, name in enumerate(self.out_names):
                d[name] = np.asarray(outs[i]).reshape(
                    self.n_cores, *self.out_avals[i].shape
                )[c]
            res.append(d)
        return res


_exec_cache = {}


def _build_exec(CH, n_chunks, D, shard_alloc, nsplit, repeats=1):
    key = (CH, n_chunks, D, shard_alloc, nsplit, repeats)
    if key in _exec_cache:
        return _exec_cache[key]
    SL = CH // 128
    nc = bacc.Bacc(None)
    rows_t = nc.dram_tensor(
        "rows", [n_chunks * CH, D], mybir.dt.bfloat16, kind="ExternalInput"
    )
    idxw_t = nc.dram_tensor(
        "idxw", [n_chunks, 128, CH // 16], mybir.dt.int16, kind="ExternalInput"
    )
    out_t = nc.dram_tensor(
        "out", [shard_alloc, D], mybir.dt.bfloat16, kind="ExternalOutput"
    )

    with (
        nc.semaphore("load_sem") as load_sem,
        nc.semaphore("scat_sem") as scat_sem,
    ):
        data_sb = [
            nc.ctx.enter_context(
                nc.sbuf_tensor(f"data{b}", [128, SL * D], mybir.dt.bfloat16)
            )
            for b in range(NB)
        ]
        idx_sb = [
            nc.ctx.enter_context(
                nc.sbuf_tensor(f"idxs{b}", [128, CH // 16], mybir.dt.int16)
            )
            for b in range(NB)
        ]

        with nc.Block() as block:

            @block.sync
            def _(sync):
                t = 0
                for _r in range(repeats):
                    for tc in range(n_chunks):
                        b = t % NB
                        if t >= NB:
                            sync.wait_ge(scat_sem, 16 * (t - NB + 1))
                        sync.dma_start(
                            out=AP(idx_sb[b], 0, [[CH // 16, 128], [1, CH // 16]]),
                            in_=AP(
                                idxw_t,
                                tc * 128 * (CH // 16),
                                [[CH // 16, 128], [1, CH // 16]],
                            ),
                        ).then_inc(load_sem, 16)
                        sync.dma_start(
                            out=AP(data_sb[b], 0, [[SL * D, 128], [1, SL * D]]),
                            in_=AP(
                                rows_t, tc * CH * D, [[SL * D, 128], [1, SL * D]]
                            ),
                        ).then_inc(load_sem, 16)
                        t += 1

            @block.gpsimd
            def _(g):
                t = 0
                for _r in range(repeats):
                    for tc in range(n_chunks):
                        b = t % NB
                        bucket = tc // nsplit
                        region, par = bucket // 2, bucket % 2
                        g.wait_ge(load_sem, 32 * (t + 1))
                        g.dma_scatter_add(
                            AP(
                                out_t,
                                (region * REGION_ROWS + par) * D,
                                [[2 * D, REGION_ROWS // 2], [1, D]],
                            ),
                            AP(data_sb[b], 0, [[SL * D, 128], [D, SL], [1, D]]),
                            AP(idx_sb[b], 0, [[CH // 16, 128], [1, CH // 16]]),
                            CH,
                            CH,
                            D,
                            elem_step=2 * D,
                        ).then_inc(scat_sem, 16)
                        t += 1
                g.wait_ge(scat_sem, 16 * t)

    nc.finalize()
    ex = SpmdExec(nc, N_CORES)
    _exec_cache[key] = ex
    return ex


def _host_prep(rows, idx, OUT):
    """Dedup + route + pack. Returns (in_maps, geom) where geom =
    (CH, n_chunks, D, shard_alloc, nsplit, SHARD)."""
    N, D = rows.shape
    SHARD = (OUT + N_CORES - 1) // N_CORES

    # ---- host routing ----
    inv = np.full(OUT, -1, dtype=np.int64)
    inv[idx] = np.arange(N)  # last occurrence wins
    win = np.flatnonzero(inv >= 0)  # sorted output rows that get written
    src = inv[win]

    core = win // SHARD
    local = win - core * SHARD
    region = local // REGION_ROWS
    rr = local - region * REGION_ROWS
    par = rr & 1
    blk = (rr >> 1).astype(np.int16)
    n_region = (SHARD + REGION_ROWS - 1) // REGION_ROWS
    n_bucket = n_region * 2
    bucket = region * 2 + par

    key = core * n_bucket + bucket
    counts = np.bincount(key, minlength=N_CORES * n_bucket).reshape(N_CORES, n_bucket)
    maxb = int(counts.max())
    nsplit = max(1, -(-maxb // CH_CAP))
    CH = max(128, min(CH_CAP, ((-(-maxb // nsplit)) + 127) // 128 * 128))
    n_chunks = n_bucket * nsplit
    SL = CH // 128

    order = np.argsort(key, kind="stable")
    k_sorted = key[order]
    blk_sorted = blk[order]
    src_sorted = src[order]
    starts = np.zeros(N_CORES * n_bucket + 1, np.int64)
    np.cumsum(counts.ravel(), out=starts[1:])
    posin = np.arange(len(win)) - starts[k_sorted]  # position within bucket
    bucket_in_core = k_sorted - (k_sorted // n_bucket) * n_bucket
    chunk_in_core = bucket_in_core * nsplit + posin // CH
    posc = posin % CH  # position within chunk
    wrap = (posc % 128) * SL + posc // 128  # wrap layout within chunk

    # Padding rows carry zero data, but their CCE add still does an HBM
    # read-modify-write: a pad colliding with a real row's address can lose
    # the real update (RMW race), and same-address descriptors serialize
    # (~96ns each). So pads target DISTINCT UNWRITTEN blocks of their
    # bucket's region half — those must be zero anyway, so +0 is harmless.
    NBLK_R = REGION_ROWS // 2
    in_maps = []
    for c in range(N_CORES):
        sel = slice(starts[c * n_bucket], starts[(c + 1) * n_bucket])
        rows_packed = np.zeros((n_chunks * CH, D), dtype=ml_dtypes.bfloat16)
        idx16 = np.empty((n_chunks, CH), dtype=np.int16)
        for b in range(n_bucket):
            s0, s1 = starts[c * n_bucket + b], starts[c * n_bucket + b + 1]
            occ = np.zeros(NBLK_R, dtype=bool)
            occ[blk_sorted[s0:s1].astype(np.int64)] = True
            un = np.flatnonzero(~occ)
            if len(un) == 0:
                un = np.arange(NBLK_R)
            fill = un[np.arange(nsplit * CH) % len(un)].astype(np.int16)
            idx16[b * nsplit : (b + 1) * nsplit] = fill.reshape(nsplit, CH)
        ci = chunk_in_core[sel]
        rows_packed[ci * CH + wrap[sel]] = rows[src_sorted[sel]]
        idx16[ci, posc[sel]] = blk_sorted[sel]
        iw = idx16.reshape(n_chunks, CH // 16, 16).transpose(0, 2, 1)
        iwf = np.ascontiguousarray(
            np.broadcast_to(iw[:, None], (n_chunks, 8, 16, CH // 16))
        ).reshape(n_chunks, 128, CH // 16)
        in_maps.append({"rows": rows_packed, "idxw": iwf})

    shard_alloc = n_region * REGION_ROWS
    return in_maps, (CH, n_chunks, D, shard_alloc, nsplit, SHARD)


def kernel(input_, indices, output_size, n_tpc):
    rows = np.asarray(input_)
    in_dtype = rows.dtype
    if rows.dtype != ml_dtypes.bfloat16:
        rows = rows.astype(ml_dtypes.bfloat16)
    idx = np.asarray(indices).astype(np.int64)
    OUT = int(output_size)

    in_maps, (CH, n_chunks, D, shard_alloc, nsplit, SHARD) = _host_prep(
        rows, idx, OUT
    )
    ex = _build_exec(CH, n_chunks, D, shard_alloc, nsplit)
    dev_in = ex.put_inputs(in_maps)
    if PROFILE:
        global LAST_HW_NS, LAST_HW_PER_CORE, LAST_TRACE_DIR
        ex.run(dev_in)  # warmup: compile + first execute outside the capture
        outs, per_core, trace_dir = _profile_run(ex, dev_in, ex.nc)
        LAST_HW_PER_CORE = per_core
        LAST_HW_NS = max(per_core.values()) if per_core else None
        LAST_TRACE_DIR = trace_dir
    else:
        outs = ex.run(dev_in)
    res = ex.results_per_core(outs)

    out_full = np.concatenate([r["out"][:SHARD] for r in res], axis=0)[:OUT]
    return np.ascontiguousarray(out_full.astype(in_dtype))

